# revision 29
# baseline (speedup 1.0000x reference)
"""Trainium2 Bass kernel for nn_ChainLVeG (latent-variable CRF energy tensor).

out[b,l,i,j,k,t,e,t2] = masked combination of two diagonal-Gaussian products
driven by 8 Linear heads of x[b,l,:].

Sharding: 8 cores = (b in 2) x (4 chunks of 16 sequence positions).
Per core:
  phase 1: all 8 Linears for its 17 tokens (16 + halo) on PE -> DRAM scratch Y
  phase 2: "cs" stage (first gaussian product), batched over 16 positions
  phase 3: per position, the big [92 x 4232] pairwise gaussian stage + DMA out

Device layout: partition p = (i_hi, j), i = i_hi*23 + i' (92 partitions);
big free dim = (i':23, k:46, t:2, t2:2) = 4232, matching DRAM output order
[i, j, k, t, t2] so output DMAs have 184-element contiguous runs.
"""

import math
import sys

sys.path.insert(0, "/opt/trn_rl_repo")

import numpy as np

import concourse.bass as bass
import concourse.tile as tile
from concourse import bacc, mybir
from concourse.bass_utils import run_bass_kernel_spmd

F32 = mybir.dt.float32
AF = mybir.ActivationFunctionType
ALU = mybir.AluOpType

B, SL, D = 2, 64, 512
NL, T, E, G = 46, 2, 1, 2
CLIP = 1.0
LOG_2PI = math.log(2.0 * math.pi)

NCORES = 8
NCHUNK = 4           # sequence chunks per batch row
NPOS = SL // NCHUNK  # 16 positions per core
NTOK = NPOS + 1      # 17 tokens (halo)

IH, IP = 2, 23       # i = ih*IP + ip
P = IH * NL          # 92 partitions = ih*46 + j
Q = NL * T           # 92 = (k, t2)
KC = 2               # k-chunks
KCW = NL // KC       # 23
TG = T * G

# feature segment offsets in the concatenated Linear output row
OFF_TCM = 0
OFF_TCV = OFF_TCM + NL * NL * T * G   # 8464
OFF_TPM = OFF_TCV + NL * NL * T * G   # 16928
OFF_TPV = OFF_TPM + NL * NL * T * G   # 25392
OFF_TW = OFF_TPV + NL * NL * T * G    # 33856
OFF_SM = OFF_TW + NL * NL * T         # 38088
OFF_SV = OFF_SM + NL * G              # 38180
OFF_SW = OFF_SV + NL * G              # 38272
FTOT = OFF_SW + NL                    # 38318

WCH = 512            # W dma / matmul chunk (columns)
OUT_POS_STRIDE = NL * NL * NL * T * T  # 389344

_cache = {}


def _dram_ap(handle_ap, elem_offset, ap_list):
    return bass.AP(
        tensor=handle_ap.tensor,
        offset=handle_ap.offset + elem_offset,
        ap=ap_list,
    )


def _build(with_bias, w_bf16):
    nc = bacc.Bacc("TRN2", target_bir_lowering=False, debug=False)

    wdt = mybir.dt.bfloat16 if w_bf16 else F32
    xin = nc.declare_dram_parameter("xT", [4, 128, NTOK], F32, isOutput=False)
    win = nc.declare_dram_parameter("Wc", [4, 128, FTOT], wdt, isOutput=False)
    bin_ = nc.declare_dram_parameter("bias", [1, FTOT], F32, isOutput=False)
    mscin = nc.declare_dram_parameter("msc", [1, NPOS * 4 + 1], F32, isOutput=False)
    oout = nc.declare_dram_parameter(
        "out", [NPOS, NL, NL, NL, T, T], F32, isOutput=True
    )
    ydram = nc.dram_tensor("Y", [NTOK, FTOT], F32)

    xap, wap, bap, mscap = xin[:], win[:], bin_[:], mscin[:]
    oap, yap = oout[:], ydram[:]

    with tile.TileContext(nc) as tc:
        from contextlib import ExitStack

        with ExitStack() as outer:
            pconst = outer.enter_context(tc.tile_pool(name="pconst", bufs=1))
            pcs = outer.enter_context(tc.tile_pool(name="pcs", bufs=1))

            # ---------------- phase 0: constants ----------------
            mmdt = wdt if w_bf16 else mybir.dt.float32r
            xt = []
            for k in range(4):
                xk = pconst.tile([128, NTOK], mmdt, name=f"xk{k}")
                src = xap[k] if w_bf16 else xap[k].bitcast(mybir.dt.float32r)
                nc.sync.dma_start(out=xk[:], in_=src)
                xt.append(xk)
            if with_bias:
                ones1 = pconst.tile([1, NTOK], F32, name="ones1")
                nc.vector.memset(ones1[:], 1.0)
                bsb = pconst.tile([1, FTOT], F32, name="bsb")
                nc.sync.dma_start(out=bsb[:], in_=bap[0:1])
            msct = pconst.tile([P, NPOS * 4 + 1], F32, name="msct")
            nc.sync.dma_start(
                out=msct[:], in_=_dram_ap(mscap, 0, [[0, P], [1, NPOS * 4 + 1]])
            )

            # keeper outputs of phase 2
            m1b = pcs.tile([P, NPOS, NL, G], F32, name="m1b")
            ub = pcs.tile([P, NPOS, NL, G], F32, name="ub")
            csb = pcs.tile([P, NPOS, NL], F32, name="csb")
            smc = pcs.tile([P, NPOS, G], F32, name="smc")
            svc = pcs.tile([P, NPOS, G], F32, name="svc")
            v1s = pcs.tile([P, NPOS, G], F32, name="v1s")
            sw_b = pcs.tile([P, NPOS], F32, name="sw_b")

            with ExitStack() as inner:
                pw = inner.enter_context(tc.tile_pool(name="pw", bufs=2))
                pstage = inner.enter_context(tc.tile_pool(name="pstage", bufs=4))
                ppsum = inner.enter_context(
                    tc.tile_pool(name="ppsum", bufs=4, space="PSUM")
                )
                pcst = inner.enter_context(tc.tile_pool(name="pcst", bufs=1))

                # ---------------- phase 1: Linears ----------------
                dma_eng = [nc.sync, nc.scalar, nc.gpsimd, nc.sync]
                nW = (FTOT + WCH - 1) // WCH
                for wi in range(nW):
                    c0 = wi * WCH
                    csz = min(WCH, FTOT - c0)
                    ps = ppsum.tile(
                        [NTOK, csz], F32, name="ps", tag="ps",
                        padded_shape=[NTOK, WCH],
                    )
                    for k in range(4):
                        wt = pw.tile(
                            [128, csz], mmdt, name=f"wt{k}", tag=f"w{k}",
                            padded_shape=[128, WCH],
                        )
                        wsrc = wap[k, :, c0 : c0 + csz]
                        if not w_bf16:
                            wsrc = wsrc.bitcast(mybir.dt.float32r)
                        dma_eng[k].dma_start(out=wt[:], in_=wsrc)
                        lhs = xt[k][:, :]
                        rhs = wt[:, :]
                        nc.tensor.matmul(
                            ps[:],
                            lhsT=lhs,
                            rhs=rhs,
                            start=(k == 0),
                            stop=(k == 3 and not with_bias),
                        )
                    if with_bias:
                        nc.tensor.matmul(
                            ps[:],
                            lhsT=ones1[:],
                            rhs=bsb[0:1, c0 : c0 + csz],
                            start=False,
                            stop=True,
                        )
                    st = pstage.tile(
                        [NTOK, csz], F32, name="st", tag="st",
                        padded_shape=[NTOK, WCH],
                    )
                    if wi % 2 == 0:
                        nc.scalar.copy(st[:], ps[:])
                    else:
                        nc.vector.tensor_copy(out=st[:], in_=ps[:])
                    nc.sync.dma_start(out=yap[:, c0 : c0 + csz], in_=st[:])

                # ------------- phase 2: cs stage (batched) -------------
                def cst(name, tag):
                    return pcst.tile([P, NPOS, NL, G], F32, name=name, tag=tag)

                tcm_b = cst("tcm_b", "gb1")
                tcv_b = cst("tcv_b", "gb2")
                geng = [nc.sync, nc.scalar, nc.gpsimd]
                gi = 0
                for t_, off in ((tcm_b, OFF_TCM), (tcv_b, OFF_TCV)):
                    for ih in range(IH):
                        for l in range(NPOS):
                            gview = t_[ih * NL : (ih + 1) * NL, l].rearrange(
                                "p (i t) g -> p i (t g)", t=T
                            )
                            geng[gi % 3].dma_start(
                                out=gview,
                                in_=_dram_ap(
                                    yap,
                                    l * FTOT + off + ih * IP * (NL * T * G),
                                    [[4, NL], [NL * T * G, IP], [1, TG]],
                                ),
                            )
                            gi += 1

                sm_b = pcst.tile([P, NPOS, G], F32, name="sm_b", tag="gs1")
                sv_b = pcst.tile([P, NPOS, G], F32, name="sv_b", tag="gs2")
                for t_, off in ((sm_b, OFF_SM), (sv_b, OFF_SV)):
                    for ih in range(IH):
                        nc.sync.dma_start(
                            out=t_[ih * NL : (ih + 1) * NL],
                            in_=_dram_ap(
                                yap, off, [[G, NL], [FTOT, NPOS], [1, G]]
                            ),
                        )
                for ih in range(IH):
                    nc.sync.dma_start(
                        out=sw_b[ih * NL : (ih + 1) * NL],
                        in_=_dram_ap(yap, OFF_SW, [[1, NL], [FTOT, NPOS]]),
                    )

                def clip_(dst, src, eng=None):
                    (eng or nc.vector).tensor_scalar(
                        dst, src, -CLIP, CLIP, ALU.max, ALU.min
                    )

                def sbc_g(t_, g):  # [P, NPOS, G] -> bcast [P, NPOS, NL] for one g
                    return t_[:, :, g].unsqueeze(2).broadcast_to([P, NPOS, NL])

                def tt_g(eng, out, in0, in1s, op):
                    # per-g split: out/in0 [P, NPOS, NL, G] tiles, in1s = g-slices
                    for g in range(G):
                        eng.tensor_tensor(
                            out[:, :, :, g], in0[:, :, :, g], in1s(g), op
                        )

                tcm_c = cst("tcm_c", "g1")
                clip_(tcm_c[:], tcm_b[:])
                tcv_c = cst("tcv_c", "g2")
                clip_(tcv_c[:], tcv_b[:])
                clip_(smc[:], sm_b[:])
                clip_(svc[:], sv_b[:])

                v2s = cst("v2s", "g3")
                nc.scalar.activation(v2s[:], tcv_c[:], AF.Exp, scale=2.0)
                nc.scalar.activation(v1s[:], svc[:], AF.Exp, scale=2.0)

                add1 = cst("add1", "g4")
                tt_g(nc.vector, add1, v2s, lambda g: sbc_g(v1s, g), ALU.add)
                la1 = cst("la1", "g5")
                nc.scalar.activation(la1[:], add1[:], AF.Ln)
                r1 = cst("r1", "g6")
                nc.vector.reciprocal_approx_fast(
                    out=r1[:].rearrange("p l i g -> p (l i g)"),
                    in_=add1[:].rearrange("p l i g -> p (l i g)"),
                )

                # cs_mu = (sm*v2s + tcm*v1s) * r1
                t1 = cst("t1", "g4")
                tt_g(nc.vector, t1, v2s, lambda g: sbc_g(smc, g), ALU.mult)
                t2 = cst("t2", "g7")
                tt_g(nc.vector, t2, tcm_c, lambda g: sbc_g(v1s, g), ALU.mult)
                t3 = cst("t3", "g8")
                nc.gpsimd.tensor_tensor(t3[:], t1[:], t2[:], ALU.add)
                nc.vector.tensor_tensor(m1b[:], t3[:], r1[:], ALU.mult)

                # cs_var = sv + tcv - 0.5*la1 ; u = exp(2*cs_var)
                svar = cst("svar", "g4")
                nc.vector.scalar_tensor_tensor(
                    svar[:], la1[:], -0.5, tcv_c[:], ALU.mult, ALU.add
                )
                svar2 = cst("svar2", "g7")
                tt_g(nc.gpsimd, svar2, svar, lambda g: sbc_g(svc, g), ALU.add)
                nc.scalar.activation(ub[:], svar2[:], AF.Exp, scale=2.0)

                # cs_scale = -0.5*(2LOG2PI + sum_g(la1 + (tcm-sm)^2 r1)) + sw
                dd = cst("dd", "g3")
                tt_g(nc.vector, dd, tcm_c, lambda g: sbc_g(smc, g), ALU.subtract)
                ee = cst("ee", "g7")
                nc.scalar.activation(ee[:], dd[:], AF.Square)
                ff = cst("ff", "g4")
                nc.vector.tensor_tensor(ff[:], ee[:], r1[:], ALU.mult)
                gg = cst("gg", "g8")
                nc.gpsimd.tensor_tensor(gg[:], ff[:], la1[:], ALU.add)
                ssum = pcst.tile([P, NPOS, NL], F32, name="ssum", tag="g7")
                nc.vector.tensor_tensor(
                    ssum[:], gg[:, :, :, 0], gg[:, :, :, 1], ALU.add
                )
                cs1 = pcst.tile([P, NPOS, NL], F32, name="cs1", tag="g8")
                nc.vector.tensor_scalar(
                    cs1[:], ssum[:], -0.5, -LOG_2PI, ALU.mult, ALU.add
                )
                nc.vector.tensor_tensor(
                    csb[:],
                    cs1[:],
                    sw_b[:].unsqueeze(2).broadcast_to([P, NPOS, NL]),
                    ALU.add,
                )

            # ---------------- phase 3: positions ----------------
            pb = outer.enter_context(tc.tile_pool(name="pb", bufs=2))
            pbig = outer.enter_context(tc.tile_pool(name="pbig", bufs=1))
            ppsR = outer.enter_context(
                tc.tile_pool(name="ppsR", bufs=1, space="PSUM")
            )
            pout = outer.enter_context(tc.tile_pool(name="pout", bufs=3))

            for l in range(NPOS):
                ytok = l + 1  # B-side token
                m2r = pb.tile([P, NL, T, G], F32, name="m2r", tag="m2r")
                wvr = pb.tile([P, NL, T, G], F32, name="wvr", tag="wvr")
                twr = pb.tile([P, Q], F32, name="twr", tag="twr")
                for ih in range(IH):
                    pr = slice(ih * NL, (ih + 1) * NL)
                    nc.sync.dma_start(
                        out=m2r[pr].rearrange("p k t2 g -> p k (t2 g)"),
                        in_=_dram_ap(
                            yap,
                            ytok * FTOT + OFF_TPM,
                            [[NL * TG, NL], [TG, NL], [1, TG]],
                        ),
                    )
                    nc.sync.dma_start(
                        out=wvr[pr].rearrange("p k t2 g -> p k (t2 g)"),
                        in_=_dram_ap(
                            yap,
                            ytok * FTOT + OFF_TPV,
                            [[NL * TG, NL], [TG, NL], [1, TG]],
                        ),
                    )
                    nc.sync.dma_start(
                        out=twr[pr],
                        in_=_dram_ap(yap, ytok * FTOT + OFF_TW, [[Q, NL], [1, Q]]),
                    )
                m2c = pb.tile([P, NL, T, G], F32, name="m2c", tag="m2c")
                clip_(m2c[:], m2r[:])
                wvc = pb.tile([P, NL, T, G], F32, name="wvc", tag="wvc")
                clip_(wvc[:], wvr[:])
                wve = pb.tile([P, NL, T, G], F32, name="wve", tag="wve")
                nc.scalar.activation(wve[:], wvc[:], AF.Exp, scale=2.0)
                twg = pb.tile([P, Q], F32, name="twg", tag="twg")
                nc.vector.tensor_scalar(
                    twg[:],
                    twr[:],
                    msct[:, 4 * l + 1 : 4 * l + 2],
                    msct[:, 4 * l + 2 : 4 * l + 3],
                    ALU.mult,
                    ALU.add,
                )

                for kc in range(KC):
                    ks = slice(kc * KCW, (kc + 1) * KCW)

                    def a_side(src, t, g):
                        # [P, IP] for fixed (l, t, g), bcast over k -> [P,IP,KCW]
                        return (
                            src[:, l, :, g]
                            .rearrange("p (i t) -> p i t", t=T)[:, :, t]
                            .unsqueeze(2)
                            .broadcast_to([P, IP, KCW])
                        )

                    def b_side(src, t2, g):
                        # [P, KCW] for fixed (t2, g), bcast over i -> [P,IP,KCW]
                        return (
                            src[:, ks, t2, g]
                            .unsqueeze(1)
                            .broadcast_to([P, IP, KCW])
                        )

                    def big(name, tag=None):
                        return pbig.tile(
                            [P, IP, KCW, T, T], F32, name=name, tag=tag or name
                        )

                    # lifetimes allow tag reuse: A:addg0->f0  B:dg0->s1
                    # C:addg1->f1  D:dg1->s2  E:Lg0->s3  F:(psum Rg0)->.
                    # G:eg0->baset H:Lg1 I:Rg1 J:eg1
                    addg0 = big("addg0", "A")
                    dg0 = big("dg0", "B")
                    addg1 = big("addg1", "C")
                    dg1 = big("dg1", "D")
                    for g, (a_, d_) in enumerate(((addg0, dg0), (addg1, dg1))):
                        for t in range(T):
                            for t2 in range(T):
                                nc.vector.tensor_tensor(
                                    a_[:, :, :, t, t2],
                                    a_side(ub, t, g),
                                    b_side(wve, t2, g),
                                    ALU.add,
                                )
                                nc.vector.tensor_tensor(
                                    d_[:, :, :, t, t2],
                                    a_side(m1b, t, g),
                                    b_side(m2c, t2, g),
                                    ALU.subtract,
                                )

                    Lg0 = big("Lg0", "E")
                    nc.scalar.activation(Lg0[:], addg0[:], AF.Ln)
                    Rg0 = ppsR.tile(
                        [P, IP, KCW, T, T], F32, name="Rg0", tag="Rg0"
                    )
                    nc.scalar.activation(Rg0[:], Lg0[:], AF.Exp, scale=-1.0)
                    eg0 = big("eg0", "Gt")
                    nc.scalar.activation(eg0[:], dg0[:], AF.Square)
                    Lg1 = big("Lg1", "H")
                    nc.scalar.activation(Lg1[:], addg1[:], AF.Ln)
                    Rg1 = big("Rg1", "I")
                    nc.scalar.activation(Rg1[:], Lg1[:], AF.Exp, scale=-1.0)
                    eg1 = big("eg1", "J")
                    nc.scalar.activation(eg1[:], dg1[:], AF.Square)

                    f0 = big("f0", "A")
                    nc.vector.tensor_tensor(f0[:], eg0[:], Rg0[:], ALU.mult)
                    f1 = big("f1", "C")
                    nc.gpsimd.tensor_tensor(f1[:], eg1[:], Rg1[:], ALU.mult)

                    s1 = big("s1", "B")
                    nc.gpsimd.tensor_tensor(s1[:], Lg0[:], Lg1[:], ALU.add)
                    s2 = big("s2", "D")
                    nc.gpsimd.tensor_tensor(s2[:], f0[:], f1[:], ALU.add)
                    s3 = big("s3", "E")
                    nc.gpsimd.tensor_tensor(s3[:], s1[:], s2[:], ALU.add)

                    # base = alpha*cs + (gamma*tw + delta)
                    baset = big("baset", "Gt")
                    for t in range(T):
                        csv = (
                            csb[:, l, :]
                            .rearrange("p (i t) -> p i t", t=T)[:, :, t]
                            .unsqueeze(2)
                            .broadcast_to([P, IP, KCW])
                        )
                        for t2 in range(T):
                            twv = (
                                twg[:]
                                .rearrange("p (k t2) -> p k t2", t2=T)[:, ks, t2]
                                .unsqueeze(1)
                                .broadcast_to([P, IP, KCW])
                            )
                            nc.vector.scalar_tensor_tensor(
                                baset[:, :, :, t, t2],
                                csv,
                                msct[:, 4 * l : 4 * l + 1],
                                twv,
                                ALU.mult,
                                ALU.add,
                            )

                    ot = pout.tile([P, IP, KCW, T, T], F32, name="ot", tag="ot")
                    nc.vector.scalar_tensor_tensor(
                        ot[:],
                        s3[:],
                        msct[:, 4 * l + 3 : 4 * l + 4],
                        baset[:],
                        ALU.mult,
                        ALU.add,
                    )

                    if l == NPOS - 1:
                        ot2 = pout.tile(
                            [P, IP, KCW, T, T], F32, name="ot2", tag="ot"
                        )
                        for t in range(T):
                            csv = (
                                csb[:, l, :]
                                .rearrange("p (i t) -> p i t", t=T)[:, :, t]
                                .unsqueeze(2)
                                .broadcast_to([P, IP, KCW])
                            )
                            for t2 in range(T):
                                nc.vector.scalar_tensor_tensor(
                                    ot2[:, :, :, t, t2],
                                    csv,
                                    msct[:, 4 * NPOS : 4 * NPOS + 1],
                                    ot[:, :, :, t, t2],
                                    ALU.mult,
                                    ALU.add,
                                )
                        ot = ot2

                    for ih in range(IH):
                        nc.sync.dma_start(
                            out=_dram_ap(
                                oap,
                                l * OUT_POS_STRIDE
                                + ih * IP * (NL * NL * T * T)
                                + kc * KCW * T * T,
                                [
                                    [NL * T * T, NL],
                                    [NL * NL * T * T, IP],
                                    [1, KCW * T * T],
                                ],
                            ),
                            in_=ot[ih * NL : (ih + 1) * NL].rearrange(
                                "j i k t t2 -> j i (k t t2)"
                            ),
                        )

    nc.compile()
    return nc


def _get_nc(with_bias, w_bf16):
    key = (with_bias, w_bf16)
    if key not in _cache:
        _cache[key] = _build(with_bias, w_bf16)
    return _cache[key]


def make_in_maps(inputs):
    x = np.asarray(inputs["x"], np.float32)
    mask = np.asarray(inputs["mask"], np.float32)
    Ws = [np.asarray(inputs[n], np.float32) for n in
          ["tcm_w", "tcv_w", "tpm_w", "tpv_w", "tw_w", "ms_w", "vs_w", "ws_w"]]
    bs = [np.asarray(inputs[n], np.float32) for n in
          ["tcm_b", "tcv_b", "tpm_b", "tpv_b", "tw_b", "ms_b", "vs_b", "ws_b"]]
    Wcat = np.concatenate(Ws, axis=1)
    bcat = np.concatenate(bs, axis=0)
    with_bias = bool(np.any(bcat != 0.0))
    w_bf16 = False
    if w_bf16:
        import ml_dtypes
        Wc = np.ascontiguousarray(
            Wcat.reshape(4, 128, FTOT).astype(ml_dtypes.bfloat16)
        )
    else:
        Wc = np.ascontiguousarray(Wcat.reshape(4, 128, FTOT))

    in_maps = []
    for c in range(NCORES):
        b = c // NCHUNK
        l0 = (c % NCHUNK) * NPOS
        xtok = np.zeros((NTOK, D), np.float32)
        ntok = min(NTOK, SL - l0)
        xtok[:ntok] = x[b, l0 : l0 + ntok]
        xT = np.ascontiguousarray(xtok.T.reshape(4, 128, NTOK))
        msc = np.zeros((NPOS * 4 + 1,), np.float32)
        for l in range(NPOS):
            gl = l0 + l
            alpha = float(mask[b, gl])
            if gl + 1 < SL:
                gamma = alpha * float(mask[b, gl + 1])
            else:
                alpha = 0.0
                gamma = 0.0
            msc[4 * l + 0] = alpha
            msc[4 * l + 1] = gamma
            msc[4 * l + 2] = -LOG_2PI * gamma
            msc[4 * l + 3] = -0.5 * gamma
        msc[4 * NPOS] = float(mask[b, SL - 1]) if l0 + NPOS == SL else 0.0
        in_maps.append(
            {
                "xT": xT,
                "Wc": Wc,
                "bias": bcat.reshape(1, FTOT).copy(),
                "msc": msc.reshape(1, -1),
            }
        )
    return in_maps, with_bias, w_bf16


def assemble(results):
    out = np.empty((B, SL, NL, NL, NL, T, E, T), np.float32)
    for c in range(NCORES):
        b = c // NCHUNK
        l0 = (c % NCHUNK) * NPOS
        oc = np.asarray(results[c]["out"]).reshape(NPOS, NL, NL, NL, T, 1, T)
        out[b, l0 : l0 + NPOS] = oc
    return out


def kernel(**inputs):
    in_maps, with_bias, w_bf16 = make_in_maps(inputs)
    nc = _get_nc(with_bias, w_bf16)
    res = run_bass_kernel_spmd(nc, in_maps, list(range(NCORES)))
    return assemble(res.results)


if __name__ == "__main__":
    print("building...")
    _get_nc(False, False)
    print("built ok")


# revision 40
# speedup vs baseline: 1.1621x; 1.1621x over previous
"""Trainium2 Bass kernel for nn_ChainLVeG (latent-variable CRF energy tensor).

out[b,l,i,j,k,t,e,t2] = masked combination of two diagonal-Gaussian products
driven by 8 Linear heads of x[b,l,:].

Sharding: 8 cores = (b in 2) x (4 chunks of 16 sequence positions).
Per core:
  phase 1: all 8 Linears for its 17 tokens (16 + halo) on PE -> DRAM scratch Y
  phase 2: "cs" stage (first gaussian product), batched over 16 positions
  phase 3: per position, the big [92 x 4232] pairwise gaussian stage + DMA out

Device layout: partition p = (i_hi, j), i = i_hi*23 + i' (92 partitions);
big free dim = (i':23, k:46, t:2, t2:2) = 4232, matching DRAM output order
[i, j, k, t, t2] so output DMAs have 184-element contiguous runs.
"""

import math
import sys

sys.path.insert(0, "/opt/trn_rl_repo")

import numpy as np

import concourse.bass as bass
import concourse.tile as tile
from concourse import bacc, mybir
from concourse.bass_utils import run_bass_kernel_spmd

F32 = mybir.dt.float32
AF = mybir.ActivationFunctionType
ALU = mybir.AluOpType

B, SL, D = 2, 64, 512
NL, T, E, G = 46, 2, 1, 2
CLIP = 1.0
LOG_2PI = math.log(2.0 * math.pi)

NCORES = 8
NCHUNK = 4           # sequence chunks per batch row
NPOS = SL // NCHUNK  # 16 positions per core
NTOK = NPOS + 1      # 17 tokens (halo)

IH, IP = 2, 23       # i = ih*IP + ip
P = IH * NL          # 92 partitions = ih*46 + j
Q = NL * T           # 92 = (k, t2)
KC = 2               # k-chunks
KCW = NL // KC       # 23
TG = T * G

# feature segment offsets in the concatenated Linear output row
OFF_TCM = 0
OFF_TCV = OFF_TCM + NL * NL * T * G   # 8464
OFF_TPM = OFF_TCV + NL * NL * T * G   # 16928
OFF_TPV = OFF_TPM + NL * NL * T * G   # 25392
OFF_TW = OFF_TPV + NL * NL * T * G    # 33856
OFF_SM = OFF_TW + NL * NL * T         # 38088
OFF_SV = OFF_SM + NL * G              # 38180
OFF_SW = OFF_SV + NL * G              # 38272
FTOT = OFF_SW + NL                    # 38318

WCH = 2048           # W dma chunk (columns)
MMN = 512            # matmul moving chunk
OUT_POS_STRIDE = NL * NL * NL * T * T  # 389344


def _tc_perm():
    """Permutation of the tc feature block: old order (i, j, t, g) ->
    new order (j, ih, i', t, g) so per-(j, ih) gathers are contiguous."""
    old = np.arange(NL * NL * T * G).reshape(NL, NL, T, G)  # [i, j, t, g]
    new = np.empty((NL, IH, IP, T, G), np.int64)
    for ih in range(IH):
        for ip in range(IP):
            new[:, ih, ip] = old[ih * IP + ip].transpose(0, 1, 2)
    return new.reshape(-1)


TC_PERM = _tc_perm()

_cache = {}


def _patch_affine_ref():
    """Make AFFINE_THEN_ADD's sim reference broadcast [P,1] scalars
    against inputs with >1 free dim."""
    from concourse import dve_ops

    def _ref(in0, in1, s0, s1, imm2):
        def bc(v):
            a = np.asarray(v)
            if a.ndim and in0.ndim > a.ndim:
                a = a.reshape(a.shape[0], *([1] * (in0.ndim - 1)))
            return a

        return (in0.astype(np.float32) * bc(s0) + bc(s1)) + in1

    object.__setattr__(dve_ops.AFFINE_THEN_ADD.spec, "reference", _ref)


_patch_affine_ref()


def _dram_ap(handle_ap, elem_offset, ap_list):
    return bass.AP(
        tensor=handle_ap.tensor,
        offset=handle_ap.offset + elem_offset,
        ap=ap_list,
    )


def _build(with_bias, w_bf16):
    nc = bacc.Bacc("TRN2", target_bir_lowering=False, debug=False)

    wdt = mybir.dt.bfloat16 if w_bf16 else F32
    xin = nc.declare_dram_parameter("xT", [4, 128, NTOK], F32, isOutput=False)
    win = nc.declare_dram_parameter("Wc", [4, 128, FTOT], wdt, isOutput=False)
    bin_ = nc.declare_dram_parameter("bias", [1, FTOT], F32, isOutput=False)
    mscin = nc.declare_dram_parameter("msc", [1, NPOS * 4 + 1], F32, isOutput=False)
    # device output layout: [l, ih, j, i', k, t, t2] — host transposes back
    oout = nc.declare_dram_parameter(
        "out", [NPOS, IH, NL, IP, NL, T, T], F32, isOutput=True
    )
    ydram = nc.dram_tensor("Y", [NTOK, FTOT], F32)

    xap, wap, bap, mscap = xin[:], win[:], bin_[:], mscin[:]
    oap, yap = oout[:], ydram[:]

    with tile.TileContext(nc) as tc:
        from contextlib import ExitStack

        with ExitStack() as outer:
            pconst = outer.enter_context(tc.tile_pool(name="pconst", bufs=1))
            pcs = outer.enter_context(tc.tile_pool(name="pcs", bufs=1))

            # ---------------- phase 0: constants ----------------
            mmdt = wdt if w_bf16 else mybir.dt.float32r
            xt = []
            for k in range(4):
                xk = pconst.tile([128, NTOK], mmdt, name=f"xk{k}")
                src = xap[k] if w_bf16 else xap[k].bitcast(mybir.dt.float32r)
                nc.sync.dma_start(out=xk[:], in_=src)
                xt.append(xk)
            if with_bias:
                ones1 = pconst.tile([1, NTOK], F32, name="ones1")
                nc.vector.memset(ones1[:], 1.0)
                bsb = pconst.tile([1, FTOT], F32, name="bsb")
                nc.sync.dma_start(out=bsb[:], in_=bap[0:1])
            msct = pconst.tile([P, NPOS * 4 + 1], F32, name="msct")
            nc.sync.dma_start(
                out=msct[:], in_=_dram_ap(mscap, 0, [[0, P], [1, NPOS * 4 + 1]])
            )

            # keeper outputs of phase 2
            m1b = pcs.tile([P, NPOS, NL, G], F32, name="m1b")
            ub = pcs.tile([P, NPOS, NL, G], F32, name="ub")
            csb = pcs.tile([P, NPOS, NL], F32, name="csb")
            smc = pcs.tile([P, NPOS, G], F32, name="smc")
            svc = pcs.tile([P, NPOS, G], F32, name="svc")
            v1s = pcs.tile([P, NPOS, G], F32, name="v1s")
            sw_b = pcs.tile([P, NPOS], F32, name="sw_b")

            with ExitStack() as inner:
                pw = inner.enter_context(tc.tile_pool(name="pw", bufs=2))
                pstage = inner.enter_context(tc.tile_pool(name="pstage", bufs=4))
                ppsum = inner.enter_context(
                    tc.tile_pool(name="ppsum", bufs=4, space="PSUM")
                )
                pcst = inner.enter_context(tc.tile_pool(name="pcst", bufs=1))

                # ---------------- phase 1: Linears ----------------
                dma_eng = [nc.sync, nc.scalar, nc.gpsimd, nc.sync]
                nW = (FTOT + WCH - 1) // WCH
                mi = 0
                for wi in range(nW):
                    c0 = wi * WCH
                    csz = min(WCH, FTOT - c0)
                    wts = []
                    for k in range(4):
                        wt = pw.tile(
                            [128, csz], mmdt, name=f"wt{k}", tag=f"w{k}",
                            padded_shape=[128, WCH],
                        )
                        wsrc = wap[k, :, c0 : c0 + csz]
                        if not w_bf16:
                            wsrc = wsrc.bitcast(mybir.dt.float32r)
                        dma_eng[k].dma_start(out=wt[:], in_=wsrc)
                        wts.append(wt)
                    m0 = 0
                    while m0 < csz:
                        msz = min(MMN, csz - m0)
                        ps = ppsum.tile(
                            [NTOK, msz], F32, name="ps", tag="ps",
                            padded_shape=[NTOK, MMN],
                        )
                        for k in range(4):
                            nc.tensor.matmul(
                                ps[:],
                                lhsT=xt[k][:, :],
                                rhs=wts[k][:, m0 : m0 + msz],
                                start=(k == 0),
                                stop=(k == 3 and not with_bias),
                            )
                        if with_bias:
                            nc.tensor.matmul(
                                ps[:],
                                lhsT=ones1[:],
                                rhs=bsb[0:1, c0 + m0 : c0 + m0 + msz],
                                start=False,
                                stop=True,
                            )
                        st = pstage.tile(
                            [NTOK, msz], F32, name="st", tag="st",
                            padded_shape=[NTOK, MMN],
                        )
                        if mi % 2 == 0:
                            nc.scalar.copy(st[:], ps[:])
                        else:
                            nc.vector.tensor_copy(out=st[:], in_=ps[:])
                        nc.sync.dma_start(
                            out=yap[:, c0 + m0 : c0 + m0 + msz], in_=st[:]
                        )
                        m0 += msz
                        mi += 1

                # ------------- phase 2: cs stage (batched) -------------
                def cst(name, tag):
                    return pcst.tile([P, NPOS, NL, G], F32, name=name, tag=tag)

                # tc features are host-permuted to (j, ih, i', t, g) order:
                # per-(j, ih) the 92 floats (i', t, g) are contiguous.
                tcm_b = cst("tcm_b", "gb1")
                tcv_b = cst("tcv_b", "gb2")
                geng = [nc.sync, nc.scalar, nc.gpsimd, nc.sync]
                gi = 0
                for t_, off in ((tcm_b, OFF_TCM), (tcv_b, OFF_TCV)):
                    for ih in range(IH):
                        gview = t_[ih * NL : (ih + 1) * NL].rearrange(
                            "p l it g -> p l (it g)"
                        )
                        geng[gi].dma_start(
                            out=gview,
                            in_=_dram_ap(
                                yap,
                                off + ih * (IP * T * G),
                                [[IH * IP * T * G, NL], [FTOT, NPOS],
                                 [1, IP * T * G]],
                            ),
                        )
                        gi += 1

                sm_b = pcst.tile([P, NPOS, G], F32, name="sm_b", tag="gs1")
                sv_b = pcst.tile([P, NPOS, G], F32, name="sv_b", tag="gs2")
                for t_, off in ((sm_b, OFF_SM), (sv_b, OFF_SV)):
                    for ih in range(IH):
                        nc.sync.dma_start(
                            out=t_[ih * NL : (ih + 1) * NL],
                            in_=_dram_ap(
                                yap, off, [[G, NL], [FTOT, NPOS], [1, G]]
                            ),
                        )
                for ih in range(IH):
                    nc.sync.dma_start(
                        out=sw_b[ih * NL : (ih + 1) * NL],
                        in_=_dram_ap(yap, OFF_SW, [[1, NL], [FTOT, NPOS]]),
                    )

                def clip_(dst, src, eng=None):
                    (eng or nc.vector).tensor_scalar(
                        dst, src, -CLIP, CLIP, ALU.max, ALU.min
                    )

                def sbc_g(t_, g):  # [P, NPOS, G] -> bcast [P, NPOS, NL] for one g
                    return t_[:, :, g].unsqueeze(2).broadcast_to([P, NPOS, NL])

                def tt_g(eng, out, in0, in1s, op):
                    # per-g split: out/in0 [P, NPOS, NL, G] tiles, in1s = g-slices
                    for g in range(G):
                        eng.tensor_tensor(
                            out[:, :, :, g], in0[:, :, :, g], in1s(g), op
                        )

                tcm_c = cst("tcm_c", "g1")
                clip_(tcm_c[:], tcm_b[:])
                tcv_c = cst("tcv_c", "g2")
                clip_(tcv_c[:], tcv_b[:])
                clip_(smc[:], sm_b[:])
                clip_(svc[:], sv_b[:])

                v2s = cst("v2s", "g3")
                nc.scalar.activation(v2s[:], tcv_c[:], AF.Exp, scale=2.0)
                nc.scalar.activation(v1s[:], svc[:], AF.Exp, scale=2.0)

                add1 = cst("add1", "g4")
                tt_g(nc.vector, add1, v2s, lambda g: sbc_g(v1s, g), ALU.add)
                la1 = cst("la1", "g5")
                nc.scalar.activation(la1[:], add1[:], AF.Ln)
                r1 = cst("r1", "g6")
                nc.vector.reciprocal_approx_fast(
                    out=r1[:].rearrange("p l i g -> p (l i g)"),
                    in_=add1[:].rearrange("p l i g -> p (l i g)"),
                )

                # cs_mu = (sm*v2s + tcm*v1s) * r1
                t1 = cst("t1", "g4")
                tt_g(nc.vector, t1, v2s, lambda g: sbc_g(smc, g), ALU.mult)
                t2 = cst("t2", "g7")
                tt_g(nc.vector, t2, tcm_c, lambda g: sbc_g(v1s, g), ALU.mult)
                t3 = cst("t3", "g8")
                nc.gpsimd.tensor_tensor(t3[:], t1[:], t2[:], ALU.add)
                nc.vector.tensor_tensor(m1b[:], t3[:], r1[:], ALU.mult)

                # cs_var = sv + tcv - 0.5*la1 ; u = exp(2*cs_var)
                svar = cst("svar", "g4")
                nc.vector.scalar_tensor_tensor(
                    svar[:], la1[:], -0.5, tcv_c[:], ALU.mult, ALU.add
                )
                svar2 = cst("svar2", "g7")
                tt_g(nc.gpsimd, svar2, svar, lambda g: sbc_g(svc, g), ALU.add)
                nc.scalar.activation(ub[:], svar2[:], AF.Exp, scale=2.0)

                # cs_scale = -0.5*(2LOG2PI + sum_g(la1 + (tcm-sm)^2 r1)) + sw
                dd = cst("dd", "g3")
                tt_g(nc.vector, dd, tcm_c, lambda g: sbc_g(smc, g), ALU.subtract)
                ee = cst("ee", "g7")
                nc.scalar.activation(ee[:], dd[:], AF.Square)
                ff = cst("ff", "g4")
                nc.vector.tensor_tensor(ff[:], ee[:], r1[:], ALU.mult)
                gg = cst("gg", "g8")
                nc.gpsimd.tensor_tensor(gg[:], ff[:], la1[:], ALU.add)
                ssum = pcst.tile([P, NPOS, NL], F32, name="ssum", tag="g7")
                nc.vector.tensor_tensor(
                    ssum[:], gg[:, :, :, 0], gg[:, :, :, 1], ALU.add
                )
                cs1 = pcst.tile([P, NPOS, NL], F32, name="cs1", tag="g8")
                nc.vector.tensor_scalar(
                    cs1[:], ssum[:], -0.5, -LOG_2PI, ALU.mult, ALU.add
                )
                nc.vector.tensor_tensor(
                    csb[:],
                    cs1[:],
                    sw_b[:].unsqueeze(2).broadcast_to([P, NPOS, NL]),
                    ALU.add,
                )

            # ---------------- phase 3: positions ----------------
            pb = outer.enter_context(tc.tile_pool(name="pb", bufs=2))
            pbig = outer.enter_context(tc.tile_pool(name="pbig", bufs=1))
            ppsR = outer.enter_context(
                tc.tile_pool(name="ppsR", bufs=1, space="PSUM")
            )
            pout = outer.enter_context(tc.tile_pool(name="pout", bufs=3))

            for l in range(NPOS):
                ytok = l + 1  # B-side token
                m2r = pb.tile([P, NL, T, G], F32, name="m2r", tag="m2r")
                wvr = pb.tile([P, NL, T, G], F32, name="wvr", tag="wvr")
                twr = pb.tile([P, Q], F32, name="twr", tag="twr")
                for ih in range(IH):
                    pr = slice(ih * NL, (ih + 1) * NL)
                    nc.sync.dma_start(
                        out=m2r[pr].rearrange("p k t2 g -> p k (t2 g)"),
                        in_=_dram_ap(
                            yap,
                            ytok * FTOT + OFF_TPM,
                            [[NL * TG, NL], [TG, NL], [1, TG]],
                        ),
                    )
                    nc.sync.dma_start(
                        out=wvr[pr].rearrange("p k t2 g -> p k (t2 g)"),
                        in_=_dram_ap(
                            yap,
                            ytok * FTOT + OFF_TPV,
                            [[NL * TG, NL], [TG, NL], [1, TG]],
                        ),
                    )
                    nc.sync.dma_start(
                        out=twr[pr],
                        in_=_dram_ap(yap, ytok * FTOT + OFF_TW, [[Q, NL], [1, Q]]),
                    )
                m2c = pb.tile([P, NL, T, G], F32, name="m2c", tag="m2c")
                clip_(m2c[:], m2r[:])
                wvc = pb.tile([P, NL, T, G], F32, name="wvc", tag="wvc")
                clip_(wvc[:], wvr[:])
                wve = pb.tile([P, NL, T, G], F32, name="wve", tag="wve")
                nc.scalar.activation(wve[:], wvc[:], AF.Exp, scale=2.0)
                twg = pb.tile([P, Q], F32, name="twg", tag="twg")
                nc.vector.tensor_scalar(
                    twg[:],
                    twr[:],
                    msct[:, 4 * l + 1 : 4 * l + 2],
                    msct[:, 4 * l + 2 : 4 * l + 3],
                    ALU.mult,
                    ALU.add,
                )

                ot = pout.tile([P, IP, NL, T, T], F32, name="ot", tag="ot")
                for kc in range(KC):
                    ks = slice(kc * KCW, (kc + 1) * KCW)

                    def a_side(src, t, g):
                        # [P, IP] for fixed (l, t, g), bcast over k -> [P,IP,KCW]
                        return (
                            src[:, l, :, g]
                            .rearrange("p (i t) -> p i t", t=T)[:, :, t]
                            .unsqueeze(2)
                            .broadcast_to([P, IP, KCW])
                        )

                    def b_side(src, t2, g):
                        # [P, KCW] for fixed (t2, g), bcast over i -> [P,IP,KCW]
                        return (
                            src[:, ks, t2, g]
                            .unsqueeze(1)
                            .broadcast_to([P, IP, KCW])
                        )

                    def big(name, tag=None):
                        return pbig.tile(
                            [P, IP, KCW, T, T], F32, name=name, tag=tag or name
                        )

                    # lifetimes allow tag reuse: A:addg0->f0  B:dg0->s1
                    # C:addg1->f1  D:dg1->s2  E:Lg0->s3  F:(psum Rg0)->.
                    # G:eg0->baset H:Lg1 I:Rg1 J:eg1
                    addg0 = big("addg0", "A")
                    dg0 = big("dg0", "B")
                    addg1 = big("addg1", "C")
                    dg1 = big("dg1", "D")
                    for g, (a_, d_) in enumerate(((addg0, dg0), (addg1, dg1))):
                        for t in range(T):
                            for t2 in range(T):
                                nc.vector.tensor_tensor(
                                    a_[:, :, :, t, t2],
                                    a_side(ub, t, g),
                                    b_side(wve, t2, g),
                                    ALU.add,
                                )
                                nc.vector.tensor_tensor(
                                    d_[:, :, :, t, t2],
                                    a_side(m1b, t, g),
                                    b_side(m2c, t2, g),
                                    ALU.subtract,
                                )

                    Lg0 = big("Lg0", "E")
                    nc.scalar.activation(Lg0[:], addg0[:], AF.Ln)
                    Rg0 = ppsR.tile(
                        [P, IP, KCW, T, T], F32, name="Rg0", tag="Rg0"
                    )
                    nc.scalar.activation(Rg0[:], Lg0[:], AF.Exp, scale=-1.0)
                    eg0 = big("eg0", "Gt")
                    nc.scalar.activation(eg0[:], dg0[:], AF.Square)
                    Lg1 = big("Lg1", "H")
                    nc.scalar.activation(Lg1[:], addg1[:], AF.Ln)
                    Rg1 = big("Rg1", "I")
                    nc.scalar.activation(Rg1[:], Lg1[:], AF.Exp, scale=-1.0)
                    eg1 = big("eg1", "J")
                    nc.scalar.activation(eg1[:], dg1[:], AF.Square)

                    f0 = big("f0", "A")
                    nc.vector.tensor_tensor(f0[:], eg0[:], Rg0[:], ALU.mult)
                    f1 = big("f1", "C")
                    nc.gpsimd.tensor_tensor(f1[:], eg1[:], Rg1[:], ALU.mult)

                    s1 = big("s1", "B")
                    nc.gpsimd.tensor_tensor(s1[:], Lg0[:], Lg1[:], ALU.add)
                    s2 = big("s2", "D")
                    nc.gpsimd.tensor_tensor(s2[:], f0[:], f1[:], ALU.add)
                    s3 = big("s3", "E")
                    nc.gpsimd.tensor_tensor(s3[:], s1[:], s2[:], ALU.add)

                    # base = alpha*cs + (gamma*tw + delta) — custom fused DVE
                    from concourse.dve_ops import AFFINE_THEN_ADD

                    baset = big("baset", "Gt")
                    for t in range(T):
                        csv = (
                            csb[:, l, :]
                            .rearrange("p (i t) -> p i t", t=T)[:, :, t]
                            .unsqueeze(2)
                            .broadcast_to([P, IP, KCW])
                        )
                        for t2 in range(T):
                            twv = (
                                twg[:]
                                .rearrange("p (k t2) -> p k t2", t2=T)[:, ks, t2]
                                .unsqueeze(1)
                                .broadcast_to([P, IP, KCW])
                            )
                            nc.vector._custom_dve(
                                AFFINE_THEN_ADD,
                                out=baset[:, :, :, t, t2],
                                in0=csv,
                                in1=twv,
                                s0=msct[:, 4 * l : 4 * l + 1],
                                s1=0.0,
                            )

                    nc.vector.scalar_tensor_tensor(
                        ot[:, :, ks, :, :],
                        s3[:],
                        msct[:, 4 * l + 3 : 4 * l + 4],
                        baset[:],
                        ALU.mult,
                        ALU.add,
                    )

                if l == NPOS - 1:
                    ot2 = pout.tile([P, IP, NL, T, T], F32, name="ot2", tag="ot")
                    for t in range(T):
                        csv = (
                            csb[:, l, :]
                            .rearrange("p (i t) -> p i t", t=T)[:, :, t]
                            .unsqueeze(2)
                            .broadcast_to([P, IP, NL])
                        )
                        for t2 in range(T):
                            nc.vector.scalar_tensor_tensor(
                                ot2[:, :, :, t, t2],
                                csv,
                                msct[:, 4 * NPOS : 4 * NPOS + 1],
                                ot[:, :, :, t, t2],
                                ALU.mult,
                                ALU.add,
                            )
                    ot = ot2

                for ih in range(IH):
                    nc.sync.dma_start(
                        out=_dram_ap(
                            oap,
                            (l * IH + ih) * (NL * IP * NL * T * T),
                            [[IP * NL * T * T, NL], [1, IP * NL * T * T]],
                        ),
                        in_=ot[ih * NL : (ih + 1) * NL].rearrange(
                            "j i k t t2 -> j (i k t t2)"
                        ),
                    )

    # Force every activation (Ln/Exp/Square/Copy) onto the one table set
    # that has them all, so the table-load pass emits a single load
    # instead of thrashing between sets.
    from concourse import hw_specs

    tabs = hw_specs.get_activation_tables(nc.m.arch)
    keep = "natural_log_exp_and_others"
    if keep in tabs:
        needed = {
            mybir.ActivationFunctionType.Ln,
            mybir.ActivationFunctionType.Exp,
            mybir.ActivationFunctionType.Square,
            mybir.ActivationFunctionType.Copy,
            mybir.ActivationFunctionType.Identity,
        }
        if needed <= tabs[keep]:
            for k in list(tabs):
                if k != keep:
                    tabs[k] = set()

    nc.compile()
    return nc


def _get_nc(with_bias, w_bf16):
    key = (with_bias, w_bf16)
    if key not in _cache:
        _cache[key] = _build(with_bias, w_bf16)
    return _cache[key]


def make_in_maps(inputs):
    x = np.asarray(inputs["x"], np.float32)
    mask = np.asarray(inputs["mask"], np.float32)
    Ws = [np.asarray(inputs[n], np.float32) for n in
          ["tcm_w", "tcv_w", "tpm_w", "tpv_w", "tw_w", "ms_w", "vs_w", "ws_w"]]
    bs = [np.asarray(inputs[n], np.float32) for n in
          ["tcm_b", "tcv_b", "tpm_b", "tpv_b", "tw_b", "ms_b", "vs_b", "ws_b"]]
    # permute the t_c blocks to (j, ih, i', t, g) feature order
    Ws[0] = Ws[0][:, TC_PERM]
    Ws[1] = Ws[1][:, TC_PERM]
    bs[0] = bs[0][TC_PERM]
    bs[1] = bs[1][TC_PERM]
    Wcat = np.concatenate(Ws, axis=1)
    bcat = np.concatenate(bs, axis=0)
    with_bias = bool(np.any(bcat != 0.0))
    w_bf16 = False
    if w_bf16:
        import ml_dtypes
        Wc = np.ascontiguousarray(
            Wcat.reshape(4, 128, FTOT).astype(ml_dtypes.bfloat16)
        )
    else:
        Wc = np.ascontiguousarray(Wcat.reshape(4, 128, FTOT))

    in_maps = []
    for c in range(NCORES):
        b = c // NCHUNK
        l0 = (c % NCHUNK) * NPOS
        xtok = np.zeros((NTOK, D), np.float32)
        ntok = min(NTOK, SL - l0)
        xtok[:ntok] = x[b, l0 : l0 + ntok]
        xT = np.ascontiguousarray(xtok.T.reshape(4, 128, NTOK))
        msc = np.zeros((NPOS * 4 + 1,), np.float32)
        for l in range(NPOS):
            gl = l0 + l
            alpha = float(mask[b, gl])
            if gl + 1 < SL:
                gamma = alpha * float(mask[b, gl + 1])
            else:
                alpha = 0.0
                gamma = 0.0
            msc[4 * l + 0] = alpha
            msc[4 * l + 1] = gamma
            msc[4 * l + 2] = -LOG_2PI * gamma
            msc[4 * l + 3] = -0.5 * gamma
        msc[4 * NPOS] = float(mask[b, SL - 1]) if l0 + NPOS == SL else 0.0
        in_maps.append(
            {
                "xT": xT,
                "Wc": Wc,
                "bias": bcat.reshape(1, FTOT).copy(),
                "msc": msc.reshape(1, -1),
            }
        )
    return in_maps, with_bias, w_bf16


def assemble(results):
    out = np.empty((B, SL, NL, NL, NL, T, E, T), np.float32)
    for c in range(NCORES):
        b = c // NCHUNK
        l0 = (c % NCHUNK) * NPOS
        oc = np.asarray(results[c]["out"]).reshape(NPOS, IH, NL, IP, NL, T, T)
        # [l, ih, j, i', k, t, t2] -> [l, (ih i'), j, k, t, t2]
        oc = oc.transpose(0, 1, 3, 2, 4, 5, 6).reshape(
            NPOS, NL, NL, NL, T, 1, T
        )
        out[b, l0 : l0 + NPOS] = oc
    return out


def kernel(**inputs):
    in_maps, with_bias, w_bf16 = make_in_maps(inputs)
    nc = _get_nc(with_bias, w_bf16)
    res = run_bass_kernel_spmd(nc, in_maps, list(range(NCORES)))
    return assemble(res.results)


if __name__ == "__main__":
    print("building...")
    _get_nc(False, False)
    print("built ok")
